# revision 13
# baseline (speedup 1.0000x reference)
"""Trainium2 Bass kernel for a 5-layer GIN (DGL AttrMasking) over a random graph.

Strategy (graph-data parallel across 8 NeuronCores):
  - Nodes are partitioned into 8 contiguous ranges at graph boundaries; each
    core owns the edges whose dst falls in its range and computes message
    aggregation + the GIN MLP for its nodes.
  - Because src endpoints are scattered over all nodes, the full node-feature
    table h is re-materialized on every core after each layer with an
    in-kernel AllGather (tables live in Shared DRAM scratchpad, bf16).
  - Edge gathers use indirect DMA (128 rows/descriptor-batch); the per-dst
    segment-sum is a one-hot selection matmul on the Tensor engine; the edge
    embedding contribution is added via a per-tile [18 x 128] count matrix
    (counts of (bond_type, bond_dir) combos per dst) times the combined
    edge-embedding table.
  - BN (eval mode) is folded into W2/bias on the host; biases are applied via
    per-partition activation bias (b1) and a broadcast add (b2').
  - Mean-pooling is a one-hot matmul per 128-node tile accumulated in SBUF,
    followed by the 300->256 head matmul; each core emits its graph range and
    the host concatenates.

All shapes/layouts below are hardcoded for N=200000, E=400000, G=1024, D=300,
L=5 with 8 cores, but the program is rebuilt per call from the actual inputs
(the instruction schedule depends only on per-tile edge-count maxima).
"""

import sys

if "/opt/trn_rl_repo" not in sys.path:
    sys.path.insert(0, "/opt/trn_rl_repo")

import numpy as np
import ml_dtypes

import concourse.bass as bass
import concourse.tile as tile
from concourse import bacc, mybir
from concourse import bass_utils

NCORES = 8
P = 128
D = 300
DP = 304  # padded feature dim for SBUF tiles
L = 5
GW = 2  # graph windows per device (device graph count <= 256)
BN_EPS = 1e-5
R_GATHER = 1  # rows-of-128 per indirect gather (HW multi-row layout TBD)
BATCH = 4  # node tiles per M1 batch
F32 = mybir.dt.float32
F32R = mybir.dt.float32r
BF16 = mybir.dt.bfloat16
I32 = mybir.dt.int32

# knobs
DEBUG_TAPS = False
TABLE_BF16 = True  # node-feature tables (and message matmuls) in bf16
MM_F32R = True  # MLP matmuls in float32r (full-rate fp32 storage)


def _cdiv(a, b):
    return -(-a // b)


class _Plan:
    pass


def _preprocess(inputs):
    pl = _Plan()
    an = np.asarray(inputs["atomic_number"]).astype(np.int64)
    ch = np.asarray(inputs["chirality_type"]).astype(np.int64)
    bt = np.asarray(inputs["bond_type"]).astype(np.int64)
    bd = np.asarray(inputs["bond_direction_type"]).astype(np.int64)
    src = np.asarray(inputs["src"]).astype(np.int64)
    dst = np.asarray(inputs["dst"]).astype(np.int64)
    gid = np.asarray(inputs["graph_ids"]).astype(np.int64)
    G = int(np.asarray(inputs["num_graphs"]))
    N = an.shape[0]
    pl.N, pl.G = N, G

    gcnt = np.bincount(gid, minlength=G)
    gstart_node = np.concatenate([[0], np.cumsum(gcnt)])
    bounds = [0]
    for dvc in range(1, NCORES):
        target = dvc * N / NCORES
        gi = int(np.searchsorted(gstart_node, target))
        if gi > 0 and abs(gstart_node[gi - 1] - target) < abs(gstart_node[gi] - target):
            gi -= 1
        bounds.append(min(max(gi, bounds[-1]), G))
    bounds.append(G)
    pl.g_lo = np.array(bounds[:-1])
    pl.g_hi = np.array(bounds[1:])
    pl.Gd = pl.g_hi - pl.g_lo
    assert (pl.Gd <= GW * P).all(), pl.Gd.max()
    node_lo = gstart_node[pl.g_lo]
    node_hi = gstart_node[pl.g_hi]
    nd = node_hi - node_lo
    pl.node_lo, pl.nd = node_lo, nd
    NT = _cdiv(int(nd.max()), P)
    NT = _cdiv(NT, BATCH) * BATCH
    NLOC = NT * P
    pl.NT, pl.NLOC = NT, NLOC
    pl.TABLE_ROWS = NCORES * NLOC

    pid_of = np.empty(N, dtype=np.int64)
    dev_of_node = np.zeros(N, dtype=np.int64)
    for dvc in range(NCORES):
        pid_of[node_lo[dvc]:node_hi[dvc]] = dvc * NLOC + np.arange(nd[dvc])
        dev_of_node[node_lo[dvc]:node_hi[dvc]] = dvc

    ed = dev_of_node[dst]
    c_e = bt * 3 + bd
    counts = np.zeros((NCORES, NT), dtype=np.int64)
    per_dev = []
    for dvc in range(NCORES):
        m = ed == dvc
        s_d, d_d, c_d = src[m], dst[m], c_e[m]
        lid = pid_of[d_d] - dvc * NLOC
        order = np.argsort(lid, kind="stable")
        s_d, lid, c_d = s_d[order], lid[order], c_d[order]
        t_d = lid // P
        counts[dvc] = np.bincount(t_d, minlength=NT)
        per_dev.append((s_d, lid, c_d, t_d))
    B = counts.max(axis=0)
    nsub_t = _cdiv(B, P)
    NSUB = int(nsub_t.sum())
    pl.nsub_t = nsub_t
    pl.NSUB = NSUB
    sub_off = np.concatenate([[0], np.cumsum(nsub_t)])
    pl.sub_off = sub_off

    idt = ml_dtypes.bfloat16 if TABLE_BF16 else np.float32
    pl.offs = np.zeros((NCORES, P, NSUB), dtype=np.int32)
    pl.dstrel = np.full((NCORES, P, NSUB), -1.0, dtype=idt)
    pl.crel = np.full((NCORES, P, NSUB), -1.0, dtype=idt)
    for dvc in range(NCORES):
        s_d, lid, c_d, t_d = per_dev[dvc]
        tile_pos = np.arange(len(t_d)) - np.concatenate(
            [[0], np.cumsum(counts[dvc])]
        )[t_d]
        col = sub_off[t_d] + tile_pos // P
        row = tile_pos % P
        pl.offs[dvc, row, col] = pid_of[s_d]
        pl.dstrel[dvc, row, col] = (lid - t_d * P).astype(np.float32)
        pl.crel[dvc, row, col] = c_d.astype(np.float32)

    pl.idx0 = np.zeros((NCORES, P, NT), dtype=np.int32)
    pl.gidw = np.full((NCORES, GW, P, NT), -1000.0, dtype=np.float32)
    for dvc in range(NCORES):
        n = int(nd[dvc])
        loc = np.arange(n)
        rows, cols = loc % P, loc // P
        orig = node_lo[dvc] + loc
        pl.idx0[dvc, rows, cols] = (an[orig] * 3 + ch[orig]).astype(np.int32)
        grel = (gid[orig] - pl.g_lo[dvc]).astype(np.float32)
        for w in range(GW):
            pl.gidw[dvc, w, rows, cols] = grel - w * P

    f32 = np.float32
    ne0 = np.asarray(inputs["node_emb0"], f32)
    ne1 = np.asarray(inputs["node_emb1"], f32)
    ee0 = np.asarray(inputs["edge_emb0"], f32)
    ee1 = np.asarray(inputs["edge_emb1"], f32)
    W1 = np.asarray(inputs["W1"], f32)
    b1 = np.asarray(inputs["b1"], f32)
    W2 = np.asarray(inputs["W2"], f32)
    b2 = np.asarray(inputs["b2"], f32)
    gam = np.asarray(inputs["bn_gamma"], f32)
    bet = np.asarray(inputs["bn_beta"], f32)
    mu = np.asarray(inputs["bn_mean"], f32)
    var = np.asarray(inputs["bn_var"], f32)
    Wd = np.asarray(inputs["Wd"], f32)
    bdd = np.asarray(inputs["bd"], f32)

    tdt = ml_dtypes.bfloat16 if TABLE_BF16 else f32
    pl.emb_comb = (ne0[:, None, :] + ne1[None, :, :]).reshape(360, D).astype(tdt)
    pl.T_all = (ee0[:, :, None, :] + ee1[:, None, :, :]).reshape(L, 18, D).astype(tdt)
    pl.W1 = W1
    b1c = np.zeros((L, P, 5), f32)
    for fc in range(5):
        w = min(P, 600 - fc * P)
        b1c[:, :w, fc] = b1[:, fc * P:fc * P + w]
    pl.b1cols = b1c
    A = gam / np.sqrt(var + BN_EPS)
    pl.W2f = W2 * A[:, None, :]
    B2 = (b2 - mu) * A + bet
    pl.B2rep = np.broadcast_to(B2[:, None, :], (L, P, D)).copy()
    wde = np.zeros((304, 256), f32)
    wde[:D] = Wd
    wde[D] = bdd
    pl.Wd_ext = wde
    pl.IOTA128 = np.broadcast_to(np.arange(P, dtype=f32), (P, P)).astype(tdt).copy()
    pl.IOTA18 = np.broadcast_to(np.arange(18, dtype=f32), (P, 18)).astype(tdt).copy()
    pl.IOTA128f = np.broadcast_to(np.arange(P, dtype=f32), (P, P)).copy()
    pl.IDENT = np.eye(P, dtype=f32)
    oz = np.zeros((P, 2), f32)
    oz[:, 0] = 1.0
    pl.ONEZ = oz
    return pl


def _build(pl):
    TDT = BF16 if TABLE_BF16 else F32
    NT, NLOC, NSUB, TR = pl.NT, pl.NLOC, pl.NSUB, pl.TABLE_ROWS
    sub_off = pl.sub_off
    KW1 = (128, 128, 44)  # k-chunks of 300
    MF1 = (128, 128, 128, 128, 88)  # f' chunks of 600

    nc = bacc.Bacc("TRN2", target_bir_lowering=False, debug=False,
                   num_devices=NCORES)

    def din(name, shape, dt):
        return nc.dram_tensor(name, list(shape), dt, kind="ExternalInput")

    offs_d = din("offs", [P, NSUB], I32)
    dstrel_d = din("dstrel", [P, NSUB], TDT)
    crel_d = din("crel", [P, NSUB], TDT)
    idx0_d = din("idx0", [P, NT], I32)
    gidw_d = din("gidw", [GW, P, NT], F32)
    embc_d = din("embc", [360, D], TDT)
    T_d = din("Tall", [L, 18, D], TDT)
    W1_d = din("W1", [L, D, 600], F32)
    b1c_d = din("b1c", [L, P, 5], F32)
    W2_d = din("W2f", [L, 600, D], F32)
    B2_d = din("B2rep", [L, P, D], F32)
    Wd_d = din("Wd_ext", [304, 256], F32)
    io128_d = din("io128", [P, P], TDT)
    io18_d = din("io18", [P, 18], TDT)
    io128f_d = din("io128f", [P, P], F32)
    onez_d = din("onez", [P, 2], F32)
    ident_d = din("ident", [P, P], F32)
    out_d = nc.dram_tensor("out", [GW * P, 256], F32, kind="ExternalOutput")
    if DEBUG_TAPS:
        dbg_t0 = nc.dram_tensor("dbg_t0", [TR, D], TDT, kind="ExternalOutput")
        dbg_t1 = nc.dram_tensor("dbg_t1", [TR, D], TDT, kind="ExternalOutput")
        dbg_gs = nc.dram_tensor("dbg_gs", [GW * P, DP], F32, kind="ExternalOutput")

    MDT = F32R if MM_F32R else F32

    from contextlib import ExitStack

    with tile.TileContext(nc) as tc, ExitStack() as ctx:
        cp = ctx.enter_context(tc.tile_pool(name="const", bufs=1))
        dram = ctx.enter_context(tc.tile_pool(name="dram", bufs=1, space="DRAM"))
        msbp = ctx.enter_context(tc.tile_pool(name="msb", bufs=3))
        sp = ctx.enter_context(tc.tile_pool(name="sp", bufs=6))
        asbp = ctx.enter_context(tc.tile_pool(name="asb", bufs=2))
        aggp = ctx.enter_context(tc.tile_pool(name="agg", bufs=2))
        h1p = ctx.enter_context(tc.tile_pool(name="h1", bufs=2))
        hsbp = ctx.enter_context(tc.tile_pool(name="hsb", bufs=2))
        psA = ctx.enter_context(tc.tile_pool(name="psA", bufs=2, space="PSUM"))
        psT = ctx.enter_context(tc.tile_pool(name="psT", bufs=1, space="PSUM"))
        psM = ctx.enter_context(tc.tile_pool(name="psM", bufs=1, space="PSUM"))
        if True:
            # ---- resident constants
            def cload(name, dr_ap, shape, dt):
                t = cp.tile(shape, dt, name=name, tag=name)
                nc.sync.dma_start(out=t[:], in_=dr_ap)
                return t

            offs_sb = cload("offs_sb", offs_d[:, :], [P, NSUB], I32)
            dstrel_sb = cload("dstrel_sb", dstrel_d[:, :], [P, NSUB], TDT)
            crel_sb = cload("crel_sb", crel_d[:, :], [P, NSUB], TDT)
            gidw_sb = [cload(f"gidw_sb{w}", gidw_d[w], [P, NT], F32)
                       for w in range(GW)]
            io128_sb = cload("io128_sb", io128_d[:, :], [P, P], TDT)
            io18_sb = cload("io18_sb", io18_d[:, :], [P, 18], TDT)
            io128f_sb = cload("io128f_sb", io128f_d[:, :], [P, P], F32)
            ident_sb = cload("ident_sb", ident_d[:, :], [P, P], F32)
            onez_sb = cload("onez_sb", onez_d[:, :], [P, 2], F32)
            T_sb = [cload(f"T_sb{l}", T_d[l], [18, D], TDT) for l in range(L)]
            def wload(name, dr_ap, shape):
                if not MM_F32R:
                    return cload(name, dr_ap, shape, F32)
                stage = sp.tile(shape, F32, name=name + "_st", tag="wstage")
                nc.sync.dma_start(out=stage[:], in_=dr_ap)
                t = cp.tile(shape, MDT, name=name, tag=name)
                nc.vector.tensor_copy(out=t[:], in_=stage[:])
                return t

            W1_sb = [[wload(f"W1_sb{l}_{kc}",
                            W1_d[l, kc * 128:kc * 128 + KW1[kc], :],
                            [KW1[kc], 600])
                      for kc in range(3)] for l in range(L)]
            b1_sb = [cload(f"b1_sb{l}", b1c_d[l], [P, 5], F32) for l in range(L)]
            W2_sb = [[wload(f"W2_sb{l}_{kc}",
                            W2_d[l, kc * 128:kc * 128 + MF1[kc], :],
                            [MF1[kc], D])
                      for kc in range(5)] for l in range(L)]
            B2_sb = [cload(f"B2_sb{l}", B2_d[l], [P, D], F32) for l in range(L)]
            Wd_sb = [cload(f"Wd_sb{kc}", Wd_d[kc * 128:kc * 128 + kw, :],
                           [kw, 256], F32)
                     for kc, kw in enumerate((128, 128, 48))]
            GS_sb = [cp.tile([P, DP], F32, name=f"GS_sb{w}", tag=f"GS_sb{w}") for w in range(GW)]
            for w in range(GW):
                nc.vector.memset(GS_sb[w][:], 0.0)

            # ---- DRAM tables (Shared for AllGather outputs) and slices
            tables = [dram.tile([TR, D], TDT, addr_space="Shared", name=f"table{i}", tag=f"table{i}")
                      for i in range(L)]
            slices = [dram.tile([NLOC, D], TDT, name=f"slice{i}", tag=f"slice{i}") for i in range(2)]
            RG = [list(range(NCORES))]

            def slice_block_view(sl, base_tile, ntiles):
                # rows [base_tile*P, ...+ntiles*P) of slice viewed as [P, ntiles, D]
                return sl[base_tile * P:(base_tile + ntiles) * P, :].rearrange(
                    "(t p) d -> p t d", p=P)

            # ---- h0: gather emb_comb rows into slice 0
            idx0_sb = cload("idx0_sb", idx0_d[:, :], [P, NT], I32)
            for g0 in range(NT):
                mt = msbp.tile([P, D], TDT, name="mt", tag="msb")
                nc.gpsimd.indirect_dma_start(
                    out=mt[:, :], out_offset=None,
                    in_=embc_d[:, :],
                    in_offset=bass.IndirectOffsetOnAxis(
                        ap=idx0_sb[:, g0:g0 + 1], axis=0))
                nc.sync.dma_start(out=slices[0][g0 * P:(g0 + 1) * P, :],
                                  in_=mt[:, :])
            nc.gpsimd.collective_compute(
                "AllGather", mybir.AluOpType.bypass,
                ins=[slices[0][:].opt()], outs=[tables[0][:].opt()],
                replica_groups=RG)
            if DEBUG_TAPS:
                nc.sync.dma_start(out=dbg_t0[:, :], in_=tables[0][:, :])

            # ---- layers
            for l in range(L):
                table_in = tables[l]
                sl_out = slices[(l + 1) % 2]
                last = l == L - 1
                # gather state
                def msg(q):
                    mt = msbp.tile([P, D], TDT, name="mt", tag="msb")
                    nc.gpsimd.indirect_dma_start(
                        out=mt[:, :], out_offset=None,
                        in_=table_in[:, :],
                        in_offset=bass.IndirectOffsetOnAxis(
                            ap=offs_sb[:, q:q + 1], axis=0))
                    return mt[:, :]

                for b0 in range(0, NT, BATCH):
                    aggT = [aggp.tile([P, BATCH * P], MDT, name=f"aggT{kc}", tag=f"agg{kc}")
                            for kc in range(3)]
                    for tb in range(BATCH):
                        t = b0 + tb
                        nq = int(pl.nsub_t[t])
                        asb = asbp.tile([P, DP], F32, tag="asb")
                        if nq == 0:
                            nc.vector.memset(asb[:, :], 0.0)
                        else:
                            Aps = psA.tile([P, D], F32, tag="psA")
                            CTps = psT.tile([P, P], F32, tag="psT")
                            for j, q in enumerate(range(sub_off[t], sub_off[t + 1])):
                                mq = msg(q)
                                S = sp.tile([P, P], TDT, tag="S")
                                nc.vector.tensor_tensor(
                                    out=S[:],
                                    in0=dstrel_sb[:, q:q + 1].to_broadcast([P, P]),
                                    in1=io128_sb[:], op=mybir.AluOpType.is_equal)
                                nc.tensor.matmul(out=Aps[:], lhsT=S[:], rhs=mq,
                                                 start=(j == 0), stop=False)
                                OH = sp.tile([P, 18], TDT, tag="OH")
                                nc.vector.tensor_tensor(
                                    out=OH[:],
                                    in0=crel_sb[:, q:q + 1].to_broadcast([P, 18]),
                                    in1=io18_sb[:], op=mybir.AluOpType.is_equal)
                                nc.tensor.matmul(out=CTps[:18, :], lhsT=OH[:],
                                                 rhs=S[:], start=(j == 0),
                                                 stop=(j == nq - 1))
                            CT = sp.tile([18, P], TDT, tag="CT")
                            nc.vector.tensor_copy(out=CT[:], in_=CTps[:18, :])
                            nc.tensor.matmul(out=Aps[:], lhsT=CT[:], rhs=T_sb[l][:],
                                             start=False, stop=True)
                            nc.vector.tensor_copy(out=asb[:, :D], in_=Aps[:])
                        # transposes -> aggT columns
                        for kc in range(3):
                            w = KW1[kc]
                            Tp = psT.tile([P, P], F32, tag="psT")
                            nc.tensor.transpose(
                                out=Tp[:w, :], in_=asb[:, kc * 128:kc * 128 + w],
                                identity=ident_sb[:])
                            nc.vector.tensor_copy(
                                out=aggT[kc][:w, tb * P:(tb + 1) * P], in_=Tp[:w, :])
                    # M1 over the batch
                    H1 = psM.tile([P, 5 * 512], F32, tag="psM")
                    h1t = [h1p.tile([MF1[fc], 512], MDT, name=f"h1t{fc}", tag=f"h1_{fc}")
                           for fc in range(5)]
                    for fc in range(5):
                        mf = MF1[fc]
                        for kc in range(3):
                            nc.tensor.matmul(
                                out=H1[:mf, fc * 512:fc * 512 + 512],
                                lhsT=W1_sb[l][kc][:, fc * 128:fc * 128 + mf],
                                rhs=aggT[kc][:KW1[kc], :],
                                start=(kc == 0), stop=(kc == 2))
                        nc.scalar.activation(
                            out=h1t[fc][:], in_=H1[:mf, fc * 512:fc * 512 + 512],
                            func=mybir.ActivationFunctionType.Relu,
                            bias=b1_sb[l][:mf, fc:fc + 1])
                    # M2 per tile
                    hs = hsbp.tile([P, BATCH, DP], MDT if last else TDT,
                                   tag="hs_f" if last else "hs_t")
                    for tb in range(BATCH):
                        t = b0 + tb
                        H2 = psA.tile([P, D], F32, tag="psA")
                        for kc in range(5):
                            nc.tensor.matmul(
                                out=H2[:],
                                lhsT=h1t[kc][:, tb * P:(tb + 1) * P],
                                rhs=W2_sb[l][kc][:],
                                start=(kc == 0), stop=(kc == 4))
                        nc.vector.tensor_tensor(out=hs[:, tb, :D], in0=H2[:],
                                                in1=B2_sb[l][:],
                                                op=mybir.AluOpType.add)
                        if not last:
                            nc.scalar.activation(
                                out=hs[:, tb, :D], in_=hs[:, tb, :D],
                                func=mybir.ActivationFunctionType.Relu)
                        else:
                            nc.vector.tensor_copy(out=hs[:, tb, D:D + 2],
                                                  in_=onez_sb[:])
                            for w in range(GW):
                                Pw = sp.tile([P, P], MDT, tag="Pw")
                                nc.vector.tensor_tensor(
                                    out=Pw[:],
                                    in0=gidw_sb[w][:, t:t + 1].to_broadcast([P, P]),
                                    in1=io128f_sb[:], op=mybir.AluOpType.is_equal)
                                Gp = psA.tile([P, D + 2], F32, tag="psA")
                                nc.tensor.matmul(out=Gp[:], lhsT=Pw[:],
                                                 rhs=hs[:, tb, :D + 2],
                                                 start=True, stop=True)
                                nc.vector.tensor_tensor(
                                    out=GS_sb[w][:, :D + 1], in0=GS_sb[w][:, :D + 1],
                                    in1=Gp[:, :D + 1], op=mybir.AluOpType.add)
                    if not last:
                        nc.sync.dma_start(
                            out=slice_block_view(sl_out, b0, BATCH),
                            in_=hs[:, :, :D])
                if not last:
                    nc.gpsimd.collective_compute(
                        "AllGather", mybir.AluOpType.bypass,
                        ins=[sl_out[:].opt()],
                        outs=[tables[l + 1][:].opt()],
                        replica_groups=RG)
                    if DEBUG_TAPS and l == 0:
                        nc.sync.dma_start(out=dbg_t1[:, :], in_=tables[1][:, :])

            # ---- pooling finalize + head
            if DEBUG_TAPS:
                for w in range(GW):
                    nc.sync.dma_start(out=dbg_gs[w * P:(w + 1) * P, :],
                                      in_=GS_sb[w][:])
            for w in range(GW):
                cnt = sp.tile([P, 1], F32, tag="cnt")
                nc.vector.tensor_scalar_max(out=cnt[:], in0=GS_sb[w][:, D:D + 1],
                                            scalar1=1.0)
                rec = sp.tile([P, 1], F32, tag="rec")
                nc.vector.reciprocal(out=rec[:], in_=cnt[:])
                gavg = asbp.tile([P, DP], F32, tag="asb")
                nc.vector.tensor_scalar_mul(out=gavg[:, :D], in0=GS_sb[w][:, :D],
                                            scalar1=rec[:, :1])
                nc.vector.memset(gavg[:, D:D + 1], 1.0)
                nc.vector.memset(gavg[:, D + 1:DP], 0.0)
                Op = psA.tile([P, 256], F32, tag="psA")
                for kc, kw in enumerate((128, 128, 48)):
                    Tp = psT.tile([P, P], F32, tag="psT")
                    nc.tensor.transpose(out=Tp[:kw, :],
                                        in_=gavg[:, kc * 128:kc * 128 + kw],
                                        identity=ident_sb[:])
                    gT = sp.tile([P, P], F32, tag="gT")
                    nc.vector.tensor_copy(out=gT[:kw, :], in_=Tp[:kw, :])
                    nc.tensor.matmul(out=Op[:], lhsT=gT[:kw, :], rhs=Wd_sb[kc][:],
                                     start=(kc == 0), stop=(kc == 2))
                osb = sp.tile([P, 256], F32, tag="osb")
                nc.vector.tensor_copy(out=osb[:], in_=Op[:])
                nc.sync.dma_start(out=out_d[w * P:(w + 1) * P, :], in_=osb[:])

    nc.compile()
    return nc


_CACHE = {}


def kernel(**inputs):
    key = b"".join(
        np.ascontiguousarray(np.asarray(inputs[k])).tobytes()
        for k in ("src", "dst", "graph_ids"))
    import hashlib
    key = hashlib.sha256(key).hexdigest()
    if key in _CACHE:
        pl, nc = _CACHE[key]
    else:
        pl = _preprocess(inputs)
        nc = _build(pl)
        _CACHE[key] = (pl, nc)

    in_maps = _in_maps(pl)
    res = bass_utils.run_bass_kernel_spmd(nc, in_maps,
                                          core_ids=list(range(NCORES)))
    full = np.zeros((pl.G, 256), np.float32)
    for dvc in range(NCORES):
        gl, gh = int(pl.g_lo[dvc]), int(pl.g_hi[dvc])
        full[gl:gh] = res.results[dvc]["out"][:gh - gl]
    return full


def _in_maps(pl):
    in_maps = []
    for dvc in range(NCORES):
        in_maps.append({
            "offs": pl.offs[dvc],
            "dstrel": np.asarray(pl.dstrel[dvc]),
            "crel": np.asarray(pl.crel[dvc]),
            "idx0": pl.idx0[dvc],
            "gidw": pl.gidw[dvc],
            "embc": np.asarray(pl.emb_comb),
            "Tall": np.asarray(pl.T_all),
            "W1": pl.W1,
            "b1c": pl.b1cols,
            "W2f": pl.W2f,
            "B2rep": pl.B2rep,
            "Wd_ext": pl.Wd_ext,
            "io128": np.asarray(pl.IOTA128),
            "io18": np.asarray(pl.IOTA18),
            "io128f": pl.IOTA128f,
            "ident": pl.IDENT,
            "onez": pl.ONEZ,
        })
    return in_maps


# revision 14
# speedup vs baseline: 1.4226x; 1.4226x over previous
"""Trainium2 Bass kernel for a 5-layer GIN (DGL AttrMasking) over a random graph.

Strategy (graph-data parallel across 8 NeuronCores):
  - Nodes are partitioned into 8 contiguous ranges at graph boundaries; each
    core owns the edges whose dst falls in its range and computes message
    aggregation + the GIN MLP for its nodes.
  - Because src endpoints are scattered over all nodes, the full node-feature
    table h is re-materialized on every core after each layer with an
    in-kernel AllGather (tables live in Shared DRAM scratchpad, bf16).
  - Edge gathers use indirect DMA (128 rows/descriptor-batch); the per-dst
    segment-sum is a one-hot selection matmul on the Tensor engine; the edge
    embedding contribution is added via a per-tile [18 x 128] count matrix
    (counts of (bond_type, bond_dir) combos per dst) times the combined
    edge-embedding table.
  - BN (eval mode) is folded into W2/bias on the host; biases are applied via
    per-partition activation bias (b1) and a broadcast add (b2').
  - Mean-pooling is a one-hot matmul per 128-node tile accumulated in SBUF,
    followed by the 300->256 head matmul; each core emits its graph range and
    the host concatenates.

All shapes/layouts below are hardcoded for N=200000, E=400000, G=1024, D=300,
L=5 with 8 cores, but the program is rebuilt per call from the actual inputs
(the instruction schedule depends only on per-tile edge-count maxima).
"""

import sys

if "/opt/trn_rl_repo" not in sys.path:
    sys.path.insert(0, "/opt/trn_rl_repo")

import numpy as np
import ml_dtypes

import concourse.bass as bass
import concourse.tile as tile
from concourse import bacc, mybir
from concourse import bass_utils

NCORES = 8
P = 128
D = 300
DP = 304  # padded feature dim for SBUF tiles
L = 5
GW = 2  # graph windows per device (device graph count <= 256)
BN_EPS = 1e-5
R_GATHER = 1  # rows-of-128 per indirect gather (HW multi-row layout TBD)
BATCH = 4  # node tiles per M1 batch
F32 = mybir.dt.float32
F32R = mybir.dt.float32r
BF16 = mybir.dt.bfloat16
I32 = mybir.dt.int32

# knobs
DEBUG_TAPS = False
TABLE_BF16 = True  # node-feature tables (and message matmuls) in bf16
MM_F32R = True  # MLP matmuls in float32r (full-rate fp32 storage)


def _cdiv(a, b):
    return -(-a // b)


class _Plan:
    pass


def _preprocess(inputs):
    pl = _Plan()
    an = np.asarray(inputs["atomic_number"]).astype(np.int64)
    ch = np.asarray(inputs["chirality_type"]).astype(np.int64)
    bt = np.asarray(inputs["bond_type"]).astype(np.int64)
    bd = np.asarray(inputs["bond_direction_type"]).astype(np.int64)
    src = np.asarray(inputs["src"]).astype(np.int64)
    dst = np.asarray(inputs["dst"]).astype(np.int64)
    gid = np.asarray(inputs["graph_ids"]).astype(np.int64)
    G = int(np.asarray(inputs["num_graphs"]))
    N = an.shape[0]
    pl.N, pl.G = N, G

    gcnt = np.bincount(gid, minlength=G)
    gstart_node = np.concatenate([[0], np.cumsum(gcnt)])
    bounds = [0]
    for dvc in range(1, NCORES):
        target = dvc * N / NCORES
        gi = int(np.searchsorted(gstart_node, target))
        if gi > 0 and abs(gstart_node[gi - 1] - target) < abs(gstart_node[gi] - target):
            gi -= 1
        bounds.append(min(max(gi, bounds[-1]), G))
    bounds.append(G)
    pl.g_lo = np.array(bounds[:-1])
    pl.g_hi = np.array(bounds[1:])
    pl.Gd = pl.g_hi - pl.g_lo
    assert (pl.Gd <= GW * P).all(), pl.Gd.max()
    node_lo = gstart_node[pl.g_lo]
    node_hi = gstart_node[pl.g_hi]
    nd = node_hi - node_lo
    pl.node_lo, pl.nd = node_lo, nd
    NT = _cdiv(int(nd.max()), P)
    NT = _cdiv(NT, BATCH) * BATCH
    NLOC = NT * P
    pl.NT, pl.NLOC = NT, NLOC
    pl.TABLE_ROWS = NCORES * NLOC

    indeg = np.bincount(dst, minlength=N)
    pid_of = np.empty(N, dtype=np.int64)
    dev_of_node = np.zeros(N, dtype=np.int64)
    for dvc in range(NCORES):
        lo, hi = node_lo[dvc], node_hi[dvc]
        # local slot = rank of node by descending in-degree (stable) so that
        # per-tile edge counts are nearly equal across cores
        order = np.argsort(-indeg[lo:hi], kind="stable")
        slot_of = np.empty(hi - lo, dtype=np.int64)
        slot_of[order] = np.arange(hi - lo)
        pid_of[lo:hi] = dvc * NLOC + slot_of
        dev_of_node[lo:hi] = dvc

    ed = dev_of_node[dst]
    c_e = bt * 3 + bd
    counts = np.zeros((NCORES, NT), dtype=np.int64)
    per_dev = []
    for dvc in range(NCORES):
        m = ed == dvc
        s_d, d_d, c_d = src[m], dst[m], c_e[m]
        lid = pid_of[d_d] - dvc * NLOC
        order = np.argsort(lid, kind="stable")
        s_d, lid, c_d = s_d[order], lid[order], c_d[order]
        t_d = lid // P
        counts[dvc] = np.bincount(t_d, minlength=NT)
        per_dev.append((s_d, lid, c_d, t_d))
    B = counts.max(axis=0)
    nsub_t = _cdiv(B, P)
    NSUB = int(nsub_t.sum())
    pl.nsub_t = nsub_t
    pl.NSUB = NSUB
    sub_off = np.concatenate([[0], np.cumsum(nsub_t)])
    pl.sub_off = sub_off

    idt = ml_dtypes.bfloat16 if TABLE_BF16 else np.float32
    pl.offs = np.zeros((NCORES, P, NSUB), dtype=np.int32)
    pl.dstrel = np.full((NCORES, P, NSUB), -1.0, dtype=idt)
    pl.crel = np.full((NCORES, P, NSUB), -1.0, dtype=idt)
    for dvc in range(NCORES):
        s_d, lid, c_d, t_d = per_dev[dvc]
        tile_pos = np.arange(len(t_d)) - np.concatenate(
            [[0], np.cumsum(counts[dvc])]
        )[t_d]
        col = sub_off[t_d] + tile_pos // P
        row = tile_pos % P
        pl.offs[dvc, row, col] = pid_of[s_d]
        pl.dstrel[dvc, row, col] = (lid - t_d * P).astype(np.float32)
        pl.crel[dvc, row, col] = c_d.astype(np.float32)

    pl.idx0 = np.zeros((NCORES, P, NT), dtype=np.int32)
    pl.gidw = np.full((NCORES, GW, P, NT), -1000.0, dtype=np.float32)
    for dvc in range(NCORES):
        n = int(nd[dvc])
        orig = node_lo[dvc] + np.arange(n)
        loc = pid_of[orig] - dvc * NLOC  # permuted local slot
        rows, cols = loc % P, loc // P
        pl.idx0[dvc, rows, cols] = (an[orig] * 3 + ch[orig]).astype(np.int32)
        grel = (gid[orig] - pl.g_lo[dvc]).astype(np.float32)
        for w in range(GW):
            pl.gidw[dvc, w, rows, cols] = grel - w * P

    f32 = np.float32
    ne0 = np.asarray(inputs["node_emb0"], f32)
    ne1 = np.asarray(inputs["node_emb1"], f32)
    ee0 = np.asarray(inputs["edge_emb0"], f32)
    ee1 = np.asarray(inputs["edge_emb1"], f32)
    W1 = np.asarray(inputs["W1"], f32)
    b1 = np.asarray(inputs["b1"], f32)
    W2 = np.asarray(inputs["W2"], f32)
    b2 = np.asarray(inputs["b2"], f32)
    gam = np.asarray(inputs["bn_gamma"], f32)
    bet = np.asarray(inputs["bn_beta"], f32)
    mu = np.asarray(inputs["bn_mean"], f32)
    var = np.asarray(inputs["bn_var"], f32)
    Wd = np.asarray(inputs["Wd"], f32)
    bdd = np.asarray(inputs["bd"], f32)

    tdt = ml_dtypes.bfloat16 if TABLE_BF16 else f32
    pl.emb_comb = (ne0[:, None, :] + ne1[None, :, :]).reshape(360, D).astype(tdt)
    pl.T_all = (ee0[:, :, None, :] + ee1[:, None, :, :]).reshape(L, 18, D).astype(tdt)
    pl.W1 = W1
    b1c = np.zeros((L, P, 5), f32)
    for fc in range(5):
        w = min(P, 600 - fc * P)
        b1c[:, :w, fc] = b1[:, fc * P:fc * P + w]
    pl.b1cols = b1c
    A = gam / np.sqrt(var + BN_EPS)
    pl.W2f = W2 * A[:, None, :]
    B2 = (b2 - mu) * A + bet
    pl.B2rep = np.broadcast_to(B2[:, None, :], (L, P, D)).copy()
    wde = np.zeros((304, 256), f32)
    wde[:D] = Wd
    wde[D] = bdd
    pl.Wd_ext = wde
    pl.IOTA128 = np.broadcast_to(np.arange(P, dtype=f32), (P, P)).astype(tdt).copy()
    pl.IOTA18 = np.broadcast_to(np.arange(18, dtype=f32), (P, 18)).astype(tdt).copy()
    pl.IOTA128f = np.broadcast_to(np.arange(P, dtype=f32), (P, P)).copy()
    pl.IDENT = np.eye(P, dtype=f32)
    oz = np.zeros((P, 2), f32)
    oz[:, 0] = 1.0
    pl.ONEZ = oz
    return pl


def _build(pl):
    TDT = BF16 if TABLE_BF16 else F32
    NT, NLOC, NSUB, TR = pl.NT, pl.NLOC, pl.NSUB, pl.TABLE_ROWS
    sub_off = pl.sub_off
    KW1 = (128, 128, 44)  # k-chunks of 300
    MF1 = (128, 128, 128, 128, 88)  # f' chunks of 600

    nc = bacc.Bacc("TRN2", target_bir_lowering=False, debug=False,
                   num_devices=NCORES)

    def din(name, shape, dt):
        return nc.dram_tensor(name, list(shape), dt, kind="ExternalInput")

    offs_d = din("offs", [P, NSUB], I32)
    dstrel_d = din("dstrel", [P, NSUB], TDT)
    crel_d = din("crel", [P, NSUB], TDT)
    idx0_d = din("idx0", [P, NT], I32)
    gidw_d = din("gidw", [GW, P, NT], F32)
    embc_d = din("embc", [360, D], TDT)
    T_d = din("Tall", [L, 18, D], TDT)
    W1_d = din("W1", [L, D, 600], F32)
    b1c_d = din("b1c", [L, P, 5], F32)
    W2_d = din("W2f", [L, 600, D], F32)
    B2_d = din("B2rep", [L, P, D], F32)
    Wd_d = din("Wd_ext", [304, 256], F32)
    io128_d = din("io128", [P, P], TDT)
    io18_d = din("io18", [P, 18], TDT)
    io128f_d = din("io128f", [P, P], F32)
    onez_d = din("onez", [P, 2], F32)
    ident_d = din("ident", [P, P], F32)
    out_d = nc.dram_tensor("out", [GW * P, 256], F32, kind="ExternalOutput")
    if DEBUG_TAPS:
        dbg_t0 = nc.dram_tensor("dbg_t0", [TR, D], TDT, kind="ExternalOutput")
        dbg_t1 = nc.dram_tensor("dbg_t1", [TR, D], TDT, kind="ExternalOutput")
        dbg_gs = nc.dram_tensor("dbg_gs", [GW * P, DP], F32, kind="ExternalOutput")

    MDT = F32R if MM_F32R else F32

    from contextlib import ExitStack

    with tile.TileContext(nc) as tc, ExitStack() as ctx:
        cp = ctx.enter_context(tc.tile_pool(name="const", bufs=1))
        dram = ctx.enter_context(tc.tile_pool(name="dram", bufs=1, space="DRAM"))
        msbp = ctx.enter_context(tc.tile_pool(name="msb", bufs=10))
        sp = ctx.enter_context(tc.tile_pool(name="sp", bufs=10))
        asbp = ctx.enter_context(tc.tile_pool(name="asb", bufs=3))
        aggp = ctx.enter_context(tc.tile_pool(name="agg", bufs=2))
        h1p = ctx.enter_context(tc.tile_pool(name="h1", bufs=2))
        hsbp = ctx.enter_context(tc.tile_pool(name="hsb", bufs=2))
        psA = ctx.enter_context(tc.tile_pool(name="psA", bufs=2, space="PSUM"))
        psT = ctx.enter_context(tc.tile_pool(name="psT", bufs=2, space="PSUM"))
        psM = ctx.enter_context(tc.tile_pool(name="psM", bufs=3, space="PSUM"))
        if True:
            # ---- resident constants
            def cload(name, dr_ap, shape, dt):
                t = cp.tile(shape, dt, name=name, tag=name)
                nc.sync.dma_start(out=t[:], in_=dr_ap)
                return t

            offs_sb = cload("offs_sb", offs_d[:, :], [P, NSUB], I32)
            dstrel_sb = cload("dstrel_sb", dstrel_d[:, :], [P, NSUB], TDT)
            crel_sb = cload("crel_sb", crel_d[:, :], [P, NSUB], TDT)
            gidw_sb = [cload(f"gidw_sb{w}", gidw_d[w], [P, NT], F32)
                       for w in range(GW)]
            io128_sb = cload("io128_sb", io128_d[:, :], [P, P], TDT)
            io18_sb = cload("io18_sb", io18_d[:, :], [P, 18], TDT)
            io128f_sb = cload("io128f_sb", io128f_d[:, :], [P, P], F32)
            ident_sb = cload("ident_sb", ident_d[:, :], [P, P], F32)
            onez_sb = cload("onez_sb", onez_d[:, :], [P, 2], F32)
            T_sb = [cload(f"T_sb{l}", T_d[l], [18, D], TDT) for l in range(L)]
            def wload(name, dr_ap, shape):
                if not MM_F32R:
                    return cload(name, dr_ap, shape, F32)
                stage = sp.tile(shape, F32, name=name + "_st", tag="wstage")
                nc.sync.dma_start(out=stage[:], in_=dr_ap)
                t = cp.tile(shape, MDT, name=name, tag=name)
                nc.vector.tensor_copy(out=t[:], in_=stage[:])
                return t

            W1_sb = [[wload(f"W1_sb{l}_{kc}",
                            W1_d[l, kc * 128:kc * 128 + KW1[kc], :],
                            [KW1[kc], 600])
                      for kc in range(3)] for l in range(L)]
            b1_sb = [cload(f"b1_sb{l}", b1c_d[l], [P, 5], F32) for l in range(L)]
            W2_sb = [[wload(f"W2_sb{l}_{kc}",
                            W2_d[l, kc * 128:kc * 128 + MF1[kc], :],
                            [MF1[kc], D])
                      for kc in range(5)] for l in range(L)]
            B2_sb = [cload(f"B2_sb{l}", B2_d[l], [P, D], F32) for l in range(L)]
            Wd_sb = [cload(f"Wd_sb{kc}", Wd_d[kc * 128:kc * 128 + kw, :],
                           [kw, 256], F32)
                     for kc, kw in enumerate((128, 128, 48))]
            GS_sb = [cp.tile([P, DP], F32, name=f"GS_sb{w}", tag=f"GS_sb{w}") for w in range(GW)]
            for w in range(GW):
                nc.vector.memset(GS_sb[w][:], 0.0)

            # ---- DRAM tables (Shared for AllGather outputs) and slices
            tables = [dram.tile([TR, D], TDT, addr_space="Shared", name=f"table{i}", tag=f"table{i}")
                      for i in range(L)]
            slices = [dram.tile([NLOC, D], TDT, name=f"slice{i}", tag=f"slice{i}") for i in range(2)]
            RG = [list(range(NCORES))]

            def slice_block_view(sl, base_tile, ntiles):
                # rows [base_tile*P, ...+ntiles*P) of slice viewed as [P, ntiles, D]
                return sl[base_tile * P:(base_tile + ntiles) * P, :].rearrange(
                    "(t p) d -> p t d", p=P)

            # ---- h0: gather emb_comb rows into slice 0
            idx0_sb = cload("idx0_sb", idx0_d[:, :], [P, NT], I32)
            for g0 in range(NT):
                mt = msbp.tile([P, D], TDT, name="mt", tag="msb")
                nc.gpsimd.indirect_dma_start(
                    out=mt[:, :], out_offset=None,
                    in_=embc_d[:, :],
                    in_offset=bass.IndirectOffsetOnAxis(
                        ap=idx0_sb[:, g0:g0 + 1], axis=0))
                nc.sync.dma_start(out=slices[0][g0 * P:(g0 + 1) * P, :],
                                  in_=mt[:, :])
            nc.gpsimd.collective_compute(
                "AllGather", mybir.AluOpType.bypass,
                ins=[slices[0][:].opt()], outs=[tables[0][:].opt()],
                replica_groups=RG)
            if DEBUG_TAPS:
                nc.sync.dma_start(out=dbg_t0[:, :], in_=tables[0][:, :])

            # ---- layers
            for l in range(L):
                table_in = tables[l]
                sl_out = slices[(l + 1) % 2]
                last = l == L - 1
                # gather state
                def msg(q):
                    mt = msbp.tile([P, D], TDT, name="mt", tag="msb")
                    nc.gpsimd.indirect_dma_start(
                        out=mt[:, :], out_offset=None,
                        in_=table_in[:, :],
                        in_offset=bass.IndirectOffsetOnAxis(
                            ap=offs_sb[:, q:q + 1], axis=0))
                    return mt[:, :]

                for b0 in range(0, NT, BATCH):
                    aggT = [aggp.tile([P, BATCH * P], MDT, name=f"aggT{kc}", tag=f"agg{kc}")
                            for kc in range(3)]
                    for tb in range(BATCH):
                        t = b0 + tb
                        nq = int(pl.nsub_t[t])
                        asb = asbp.tile([P, DP], F32, tag="asb")
                        if nq == 0:
                            nc.vector.memset(asb[:, :], 0.0)
                        else:
                            Aps = psA.tile([P, D], F32, tag="psA")
                            CTps = psT.tile([P, P], F32, tag="psT")
                            for j, q in enumerate(range(sub_off[t], sub_off[t + 1])):
                                mq = msg(q)
                                S = sp.tile([P, P], TDT, tag="S")
                                nc.vector.tensor_tensor(
                                    out=S[:],
                                    in0=dstrel_sb[:, q:q + 1].to_broadcast([P, P]),
                                    in1=io128_sb[:], op=mybir.AluOpType.is_equal)
                                nc.tensor.matmul(out=Aps[:], lhsT=S[:], rhs=mq,
                                                 start=(j == 0), stop=False)
                                OH = sp.tile([P, 18], TDT, tag="OH")
                                nc.vector.tensor_tensor(
                                    out=OH[:],
                                    in0=crel_sb[:, q:q + 1].to_broadcast([P, 18]),
                                    in1=io18_sb[:], op=mybir.AluOpType.is_equal)
                                nc.tensor.matmul(out=CTps[:18, :], lhsT=OH[:],
                                                 rhs=S[:], start=(j == 0),
                                                 stop=(j == nq - 1))
                            CT = sp.tile([18, P], TDT, tag="CT")
                            nc.vector.tensor_copy(out=CT[:], in_=CTps[:18, :])
                            nc.tensor.matmul(out=Aps[:], lhsT=CT[:], rhs=T_sb[l][:],
                                             start=False, stop=True)
                            nc.vector.tensor_copy(out=asb[:, :D], in_=Aps[:])
                        # transposes -> aggT columns
                        for kc in range(3):
                            w = KW1[kc]
                            Tp = psT.tile([P, P], F32, tag="psT")
                            nc.tensor.transpose(
                                out=Tp[:w, :], in_=asb[:, kc * 128:kc * 128 + w],
                                identity=ident_sb[:])
                            nc.vector.tensor_copy(
                                out=aggT[kc][:w, tb * P:(tb + 1) * P], in_=Tp[:w, :])
                    # M1 over the batch (one psum tile per f-chunk, pipelined)
                    h1t = [h1p.tile([MF1[fc], 512], MDT, name=f"h1t{fc}", tag=f"h1_{fc}")
                           for fc in range(5)]
                    for fc in range(5):
                        mf = MF1[fc]
                        H1 = psM.tile([P, 512], F32, name="H1", tag="psM")
                        for kc in range(3):
                            nc.tensor.matmul(
                                out=H1[:mf, :],
                                lhsT=W1_sb[l][kc][:, fc * 128:fc * 128 + mf],
                                rhs=aggT[kc][:KW1[kc], :],
                                start=(kc == 0), stop=(kc == 2))
                        nc.scalar.activation(
                            out=h1t[fc][:], in_=H1[:mf, :],
                            func=mybir.ActivationFunctionType.Relu,
                            bias=b1_sb[l][:mf, fc:fc + 1])
                    # M2 per tile
                    hs = hsbp.tile([P, BATCH, DP], MDT if last else TDT,
                                   tag="hs_f" if last else "hs_t")
                    for tb in range(BATCH):
                        t = b0 + tb
                        H2 = psA.tile([P, D], F32, tag="psA")
                        for kc in range(5):
                            nc.tensor.matmul(
                                out=H2[:],
                                lhsT=h1t[kc][:, tb * P:(tb + 1) * P],
                                rhs=W2_sb[l][kc][:],
                                start=(kc == 0), stop=(kc == 4))
                        nc.vector.tensor_tensor(out=hs[:, tb, :D], in0=H2[:],
                                                in1=B2_sb[l][:],
                                                op=mybir.AluOpType.add)
                        if not last:
                            nc.scalar.activation(
                                out=hs[:, tb, :D], in_=hs[:, tb, :D],
                                func=mybir.ActivationFunctionType.Relu)
                        else:
                            nc.vector.tensor_copy(out=hs[:, tb, D:D + 2],
                                                  in_=onez_sb[:])
                            for w in range(GW):
                                Pw = sp.tile([P, P], MDT, tag="Pw")
                                nc.vector.tensor_tensor(
                                    out=Pw[:],
                                    in0=gidw_sb[w][:, t:t + 1].to_broadcast([P, P]),
                                    in1=io128f_sb[:], op=mybir.AluOpType.is_equal)
                                Gp = psA.tile([P, D + 2], F32, tag="psA")
                                nc.tensor.matmul(out=Gp[:], lhsT=Pw[:],
                                                 rhs=hs[:, tb, :D + 2],
                                                 start=True, stop=True)
                                nc.vector.tensor_tensor(
                                    out=GS_sb[w][:, :D + 1], in0=GS_sb[w][:, :D + 1],
                                    in1=Gp[:, :D + 1], op=mybir.AluOpType.add)
                    if not last:
                        nc.sync.dma_start(
                            out=slice_block_view(sl_out, b0, BATCH),
                            in_=hs[:, :, :D])
                if not last:
                    nc.gpsimd.collective_compute(
                        "AllGather", mybir.AluOpType.bypass,
                        ins=[sl_out[:].opt()],
                        outs=[tables[l + 1][:].opt()],
                        replica_groups=RG)
                    if DEBUG_TAPS and l == 0:
                        nc.sync.dma_start(out=dbg_t1[:, :], in_=tables[1][:, :])

            # ---- pooling finalize + head
            if DEBUG_TAPS:
                for w in range(GW):
                    nc.sync.dma_start(out=dbg_gs[w * P:(w + 1) * P, :],
                                      in_=GS_sb[w][:])
            for w in range(GW):
                cnt = sp.tile([P, 1], F32, tag="cnt")
                nc.vector.tensor_scalar_max(out=cnt[:], in0=GS_sb[w][:, D:D + 1],
                                            scalar1=1.0)
                rec = sp.tile([P, 1], F32, tag="rec")
                nc.vector.reciprocal(out=rec[:], in_=cnt[:])
                gavg = asbp.tile([P, DP], F32, tag="asb")
                nc.vector.tensor_scalar_mul(out=gavg[:, :D], in0=GS_sb[w][:, :D],
                                            scalar1=rec[:, :1])
                nc.vector.memset(gavg[:, D:D + 1], 1.0)
                nc.vector.memset(gavg[:, D + 1:DP], 0.0)
                Op = psA.tile([P, 256], F32, tag="psA")
                for kc, kw in enumerate((128, 128, 48)):
                    Tp = psT.tile([P, P], F32, tag="psT")
                    nc.tensor.transpose(out=Tp[:kw, :],
                                        in_=gavg[:, kc * 128:kc * 128 + kw],
                                        identity=ident_sb[:])
                    gT = sp.tile([P, P], F32, tag="gT")
                    nc.vector.tensor_copy(out=gT[:kw, :], in_=Tp[:kw, :])
                    nc.tensor.matmul(out=Op[:], lhsT=gT[:kw, :], rhs=Wd_sb[kc][:],
                                     start=(kc == 0), stop=(kc == 2))
                osb = sp.tile([P, 256], F32, tag="osb")
                nc.vector.tensor_copy(out=osb[:], in_=Op[:])
                nc.sync.dma_start(out=out_d[w * P:(w + 1) * P, :], in_=osb[:])

    nc.compile()
    return nc


_CACHE = {}


def kernel(**inputs):
    key = b"".join(
        np.ascontiguousarray(np.asarray(inputs[k])).tobytes()
        for k in ("src", "dst", "graph_ids"))
    import hashlib
    key = hashlib.sha256(key).hexdigest()
    if key in _CACHE:
        pl, nc = _CACHE[key]
    else:
        pl = _preprocess(inputs)
        nc = _build(pl)
        _CACHE[key] = (pl, nc)

    in_maps = _in_maps(pl)
    res = bass_utils.run_bass_kernel_spmd(nc, in_maps,
                                          core_ids=list(range(NCORES)))
    full = np.zeros((pl.G, 256), np.float32)
    for dvc in range(NCORES):
        gl, gh = int(pl.g_lo[dvc]), int(pl.g_hi[dvc])
        full[gl:gh] = res.results[dvc]["out"][:gh - gl]
    return full


def _in_maps(pl):
    in_maps = []
    for dvc in range(NCORES):
        in_maps.append({
            "offs": pl.offs[dvc],
            "dstrel": np.asarray(pl.dstrel[dvc]),
            "crel": np.asarray(pl.crel[dvc]),
            "idx0": pl.idx0[dvc],
            "gidw": pl.gidw[dvc],
            "embc": np.asarray(pl.emb_comb),
            "Tall": np.asarray(pl.T_all),
            "W1": pl.W1,
            "b1c": pl.b1cols,
            "W2f": pl.W2f,
            "B2rep": pl.B2rep,
            "Wd_ext": pl.Wd_ext,
            "io128": np.asarray(pl.IOTA128),
            "io18": np.asarray(pl.IOTA18),
            "io128f": pl.IOTA128f,
            "ident": pl.IDENT,
            "onez": pl.ONEZ,
        })
    return in_maps
